# revision 23
# baseline (speedup 1.0000x reference)
"""Trainium2 Bass kernel for nn_D6BPixelMotifBranch (pooling / memory-bound).

Data-parallel over batch across 8 NeuronCores; 4 groups of 8 batches per core.
Stage 1 (per group):
  - one contiguous-descriptor SWDGE cast-DMA (f32->bf16) per group with
    token permutation token = p*NT + q  (partition p holds NT consecutive
    tokens), giving 4.6KB descriptor runs (single big DMAs also minimize
    the serialization hops of the DMA-transpose deadlock guard);
  - one DMA-xbar tiled transpose per group produces hT chunks
    [2q*64d, 128 tok] directly (no PE transposes, no PSUM->SBUF copies);
  - per chunk-tile: 8 logits matmuls (lhsT = hT chunk, rhs = block-diag AT2),
    batched softmax over slots in [tok, 8b*2q*16k] layout, parity-sliced
    full-width pooling matmuls accumulate G in [8b*16k, 4b*64d] PSUM layout;
  - G extraction = pure column-select copies (no partition shifts, no DMA).
Stage 2 (per pair of groups, interleaved): post-norm transformer block +
class cross-attention, batched 8 batches x 16 slots per 128 partitions.
LayerNorm rsqrt runs on DVE via the f32 exponent bit trick (no Ln/Sqrt)
and gelu uses a Square/Tanh decomposition, so the scalar engine stays on a
single activation table (one ACT_TABLE_LOAD for the whole kernel).
"""

import sys
for _p in ("/opt/trn_rl_repo", "/root/.axon_site/_ro/trn_rl_repo"):
    if _p not in sys.path:
        sys.path.append(_p)

import numpy as np
import ml_dtypes

import concourse.bacc as bacc
import concourse.tile as tile
from concourse import mybir

BF16 = mybir.dt.bfloat16
F32 = mybir.dt.float32
I32 = mybir.dt.int32
AF = mybir.ActivationFunctionType
ALU = mybir.AluOpType

# problem dims
B, N, D, K, C, NH, HGT, WID = 256, 2304, 64, 16, 7, 4, 48, 48
DH = D // NH
FF = 4 * D
NCORES = 8

_bf = lambda x: np.ascontiguousarray(x.astype(ml_dtypes.bfloat16))
_f32 = lambda x: np.ascontiguousarray(x.astype(np.float32))


def _positions(n):
    ys = np.linspace(0.0, 1.0, HGT, dtype=np.float64)
    xs = np.linspace(0.0, 1.0, WID, dtype=np.float64)
    yy, xx = np.meshgrid(ys, xs, indexing="ij")
    pos = np.stack([xx.reshape(-1), yy.reshape(-1)], axis=-1)  # [N,2]
    return pos[:n].astype(np.float64)


class BlobPacker:
    """Pack named [128, w] host arrays into one blob per dtype."""

    def __init__(self):
        self.cols = {BF16: 0, F32: 0}
        self.items = []          # (name, dtype, off, shape, array)

    def add(self, name, arr, dt):
        a = np.asarray(arr)
        if a.ndim == 1:
            a = a[None, :]
        rows, w = a.shape[0], int(np.prod(a.shape[1:]))
        flat = np.zeros((128, w), np.float64)
        flat[:rows, :] = a.reshape(rows, w)
        off = self.cols[dt]
        self.cols[dt] = off + w
        self.items.append((name, dt, off, tuple(a.shape), flat))

    def finalize(self):
        blobs = {}
        for dt, conv in ((BF16, _bf), (F32, _f32)):
            w = max(self.cols[dt], 1)
            blob = np.zeros((128, w), np.float64)
            for name, d, off, shape, flat in self.items:
                if d is dt:
                    blob[:, off:off + flat.shape[1]] = flat
            blobs[dt] = conv(blob)
        meta = {name: (dt, off, shape) for name, dt, off, shape, _ in self.items}
        return blobs, meta


def host_prep(params, n=N):
    """Precompute all constant tensors, packed into two blobs."""
    p = {k: np.asarray(v, dtype=np.float64) for k, v in params.items()}
    scale = 1.0 / np.sqrt(np.float64(D))
    q = p["part_queries"]
    q = q / np.maximum(np.linalg.norm(q, axis=-1, keepdims=True), 1e-6)
    NT = n // 128

    pk = BlobPacker()
    flags = {}

    # slot logits: h @ AT + c0; block-diag [128, 32] for 2q (or 2b) pairing
    AT = (p["Wk_pix"] @ q.T) * scale                     # [64, 16]
    c0 = (p["bk_pix"] @ q.T) * scale                     # [16]
    AT2 = np.zeros((128, 32))
    AT2[0:64, 0:16] = AT
    AT2[64:128, 16:32] = AT
    pk.add("AT2", AT2, BF16)
    flags["c0"] = bool(np.any(c0 != 0.0))
    pk.add("expc0_b", np.tile(np.exp(c0)[None, :], (128, 1)), F32)

    # positions (x, y, 1), token-permuted: token = p*NT + q
    posE = np.concatenate([_positions(n), np.ones((n, 1))], -1)  # [n, 3]
    pk.add("posE", posE.reshape(128, NT, 3), BF16)

    # post-pool projection of raw pooled h
    pk.add("WvpixR", np.vstack([p["Wv_pix"], p["Wv_pix"]]), BF16)
    bvb2 = p["bv_pix"] + p["pos_b2"]
    flags["bvb2"] = bool(np.any(bvb2 != 0.0))
    pk.add("bvb2_b", np.tile(bvb2[None, :], (128, 1)), F32)

    # pos mlp
    pk.add("w1x_b", np.tile(p["pos_w1"][0][None, :], (128, 1)), F32)
    pk.add("w1y_b", np.tile(p["pos_w1"][1][None, :], (128, 1)), F32)
    flags["posb1"] = bool(np.any(p["pos_b1"] != 0.0))
    pk.add("posb1_b", np.tile(p["pos_b1"][None, :], (128, 1)), F32)
    pk.add("posw2R", np.vstack([p["pos_w2"], p["pos_w2"]]), BF16)

    # attention qkv with head padding: head h at partitions 32h..32h+16
    attn_scale = 1.0 / np.sqrt(np.float64(DH))
    Wq = p["Wqkv"][:, 0:D] * attn_scale
    Wk = p["Wqkv"][:, D:2 * D]
    Wv = p["Wqkv"][:, 2 * D:3 * D]
    bq = p["bqkv"][0:D] * attn_scale
    bk = p["bqkv"][D:2 * D]
    bv = p["bqkv"][2 * D:3 * D]
    Wq_pad = np.zeros((D, 128))
    Wk_pad = np.zeros((D, 128))
    bq_pad = np.zeros(128)
    bk_pad = np.zeros(128)
    for h in range(NH):
        Wq_pad[:, 32 * h:32 * h + DH] = Wq[:, DH * h:DH * (h + 1)]
        Wk_pad[:, 32 * h:32 * h + DH] = Wk[:, DH * h:DH * (h + 1)]
        bq_pad[32 * h:32 * h + DH] = bq[DH * h:DH * (h + 1)]
        bk_pad[32 * h:32 * h + DH] = bk[DH * h:DH * (h + 1)]
    pk.add("WqR", np.vstack([Wq_pad, Wq_pad]), BF16)     # [128, 128]
    pk.add("WkR", np.vstack([Wk_pad, Wk_pad]), BF16)
    pk.add("bq_pad", bq_pad[:, None], F32)               # [128, 1]
    pk.add("bk_pad", bk_pad[:, None], F32)
    pk.add("WvaR", np.vstack([Wv, Wv]), BF16)            # [128, 64]
    flags["bv_attn"] = bool(np.any(bv != 0.0))
    pk.add("bva_b", np.tile(bv[None, :], (128, 1)), F32)

    pk.add("WoR", np.vstack([p["Wo"], p["Wo"]]), BF16)
    flags["bo"] = bool(np.any(p["bo"] != 0.0))
    pk.add("bo_b", np.tile(p["bo"][None, :], (128, 1)), F32)

    flags["ln1"] = bool(np.any(p["ln1_g"] != 1.0) or np.any(p["ln1_b"] != 0.0))
    pk.add("ln1g_b", np.tile(p["ln1_g"][None, :], (128, 1)), F32)
    pk.add("ln1b_b", np.tile(p["ln1_b"][None, :], (128, 1)), F32)
    flags["ln2"] = bool(np.any(p["ln2_g"] != 1.0) or np.any(p["ln2_b"] != 0.0))
    pk.add("ln2g_b", np.tile(p["ln2_g"][None, :], (128, 1)), F32)
    pk.add("ln2b_b", np.tile(p["ln2_b"][None, :], (128, 1)), F32)

    # ffn
    W1 = p["ffn_w1"]                                     # [64, 256]
    pk.add("W1aR", np.vstack([W1[:, 0:128], W1[:, 0:128]]), BF16)
    pk.add("W1bR", np.vstack([W1[:, 128:256], W1[:, 128:256]]), BF16)
    pk.add("b1a", p["ffn_b1"][0:128, None], F32)         # [128, 1]
    pk.add("b1b", p["ffn_b1"][128:256, None], F32)
    pk.add("W2a", p["ffn_w2"][0:128, :], BF16)           # [128, 64]
    pk.add("W2b", p["ffn_w2"][128:256, :], BF16)
    flags["b2ffn"] = bool(np.any(p["ffn_b2"] != 0.0))
    pk.add("b2f_b", np.tile(p["ffn_b2"][None, :], (128, 1)), F32)

    # class cross attention
    pk.add("WkclsR", np.vstack([p["Wk_cls"], p["Wk_cls"]]), BF16)
    pk.add("bkcls", np.concatenate([p["bk_cls"], np.zeros(64)])[:, None], F32)
    pk.add("CqT", (p["class_queries"] * scale).T, BF16)  # [64, 7]
    wvec = p["Wv_cls"] @ p["Wh"]                         # [64, 1]
    pk.add("wvecR", np.vstack([wvec, wvec]), BF16)       # [128, 1]
    flags["cvh"] = float(p["bv_cls"] @ p["Wh"][:, 0])
    flags["bh"] = float(p["bh"][0])

    pk.add("ident", np.eye(128), BF16)
    # full block-diag mask: 1 iff same 16-slot (batch) block
    bd128 = np.kron(np.eye(8), np.ones((16, 16)))
    pk.add("bd128", bd128, BF16)
    # G-extraction partition masks: mask8[p, b] = 1 iff p//16 == b
    m8 = np.zeros((128, 8))
    for pp in range(128):
        m8[pp, pp // 16] = 1.0
    pk.add("mask8", m8, F32)
    # block-diag mask for zeroing cross-batch attention scores
    bd = np.zeros((128, 32))
    for pp in range(128):
        half = (pp % 32) // 16
        bd[pp, 16 * half:16 * (half + 1)] = 1.0
    pk.add("bdiag", bd, BF16)

    blobs, meta = pk.finalize()
    return blobs, meta, flags


def build(nbatch, n, use_mask, flags, blob_cols, meta):
    """Build the per-core Bass program. nbatch must be a multiple of 16."""
    NT = n // 128
    assert nbatch % 16 == 0 and n % 256 == 0 and NT % 2 == 0
    NG = nbatch // 8
    NCT = NT // 2

    nc = bacc.Bacc("TRN2", debug=False)

    hp = nc.dram_tensor("hp", [nbatch, n, D], F32, kind="ExternalInput")
    if use_mask:
        mk_d = nc.dram_tensor("mask", [nbatch, n], I32, kind="ExternalInput")
    blob_bf_d = nc.dram_tensor("blob_bf", [128, blob_cols[BF16]], BF16,
                               kind="ExternalInput")
    blob_f32_d = nc.dram_tensor("blob_f32", [128, blob_cols[F32]], F32,
                                kind="ExternalInput")
    out_d = nc.dram_tensor("out", [nbatch, C], F32, kind="ExternalOutput")

    with tile.TileContext(nc) as tc:
        _build_body(nc, tc, hp, mk_d if use_mask else None,
                    (blob_bf_d, blob_f32_d), out_d,
                    nbatch, n, NT, NG, NCT, flags, meta)
    nc.compile()
    return nc


def _build_body(nc, tc, hp, mk_d, blob_drams, out_d, nbatch, n, NT, NG, NCT,
                flags, meta):
    from contextlib import ExitStack
    import concourse.bass as bass
    ctx = ExitStack()
    with ctx:
        cst = ctx.enter_context(tc.tile_pool(name="cst", bufs=1))
        # ---- constants: two blob DMAs, then views ----
        blob_bf_d, blob_f32_d = blob_drams
        blob_bf = cst.tile(list(blob_bf_d.shape), BF16, name="blob_bf")
        blob_f32 = cst.tile(list(blob_f32_d.shape), F32, name="blob_f32")
        nc.sync.dma_start(out=blob_bf, in_=blob_bf_d.ap())
        nc.sync.dma_start(out=blob_f32, in_=blob_f32_d.ap())
        sb = {}
        for name, (dt, off, shape) in meta.items():
            blob = blob_bf if dt is BF16 else blob_f32
            w = int(np.prod(shape[1:]))
            v = blob[:shape[0], off:off + w]
            if len(shape) == 3:
                v = v.rearrange("p (a b) -> p a b", b=shape[2])
            sb[name] = v
        ones_bf = cst.tile([128, 1], BF16)
        nc.vector.memset(ones_bf, 1.0)
        eps_t = cst.tile([128, 1], F32)
        nc.vector.memset(eps_t, 1e-5)
        half_t = cst.tile([128, 1], F32)
        nc.vector.memset(half_t, 0.5)
        ones7 = cst.tile([1, C], BF16)
        nc.vector.memset(ones7, 1.0)
        # pooled stats per group: [128 (8b x 16k), 64 P | 2 C | 1 S]
        G = [cst.tile([128, 67], F32, name=f"G{g}") for g in range(NG)]
        OUT_sb = cst.tile([C, nbatch], F32)

        # ---- pools ----
        hbp = ctx.enter_context(tc.tile_pool(name="hbp", bufs=2))
        xtp = ctx.enter_context(tc.tile_pool(name="xtp", bufs=2))
        smp = ctx.enter_context(tc.tile_pool(name="smp", bufs=4))
        lgp = ctx.enter_context(tc.tile_pool(name="lgp", bufs=2, space="PSUM"))
        ppp = ctx.enter_context(tc.tile_pool(name="ppp", bufs=2, space="PSUM"))
        pop = ctx.enter_context(tc.tile_pool(name="pop", bufs=2, space="PSUM"))

        s2 = ctx.enter_context(tc.tile_pool(name="s2", bufs=2))
        s2p = ctx.enter_context(tc.tile_pool(name="s2p", bufs=2, space="PSUM"))
        e2p = ctx.enter_context(tc.tile_pool(name="e2p", bufs=4))

        # persistent stage-2 tensors
        NCHUNK = NG
        qT_sb = cst.tile([128, 128 * NCHUNK], BF16)
        kT_sb = cst.tile([128, 128 * NCHUNK], BF16)
        KclsT_sb = cst.tile([64, 128 * NCHUNK], BF16)
        wvT_sb = cst.tile([1, 128 * NCHUNK], BF16)
        V_sb = cst.tile([128, 64 * NCHUNK], BF16)
        NXP = (NCHUNK + 1) // 2
        Xpair = [cst.tile([128, 128], BF16, name=f"Xp{i}") for i in range(NXP)]
        Pbarp = [cst.tile([128, 128], BF16, name=f"Pb{i}") for i in range(NXP)]
        h1gp = [cst.tile([128, 128], BF16, name=f"h1g{i}") for i in range(NXP)]
        saNp = [cst.tile([128, 128], BF16, name=f"saN{i}") for i in range(NXP)]
        x1p_t = [cst.tile([128, 128], BF16, name=f"x1p{i}") for i in range(NXP)]
        x2p_t = [cst.tile([128, 128], BF16, name=f"x2p{i}") for i in range(NXP)]

        # ---------------- stage-2 helpers ----------------
        def xbar(dst, src):
            # PE-based [128,128] transpose (stage-2 chain-latency critical)
            ptx = s2p.tile([128, 128], F32, tag="ps", name="ptx")
            nc.tensor.matmul(ptx, src, sb["ident"], start=True, stop=True)
            nc.scalar.copy(out=dst, in_=ptx)

        def gelu(dst, src, bias=None):
            """tanh-approx gelu; ACT uses only Square/Tanh (both share the
            exp activation table -> no ACT_TABLE_LOAD thrash)."""
            P, Fr = src.shape[0], src.shape[1]
            a_c = 0.7978845608028654
            b_c = 0.044715
            if bias is not None:
                x = s2.tile([128, 128], F32, tag="gx", name="gx")[:P, :Fr]
                nc.scalar.activation(out=x, in_=src, func=AF.Identity,
                                     bias=bias)
            else:
                x = src
            sq = s2.tile([128, 128], F32, tag="gsq", name="gsq")[:P, :Fr]
            nc.scalar.activation(out=sq, in_=x, func=AF.Square)
            v = s2.tile([128, 128], F32, tag="gv", name="gv")[:P, :Fr]
            nc.vector.tensor_scalar(v, sq, a_c * b_c, a_c,
                                    op0=ALU.mult, op1=ALU.add)
            u = s2.tile([128, 128], F32, tag="gu", name="gu")[:P, :Fr]
            nc.vector.tensor_tensor(out=u, in0=x, in1=v, op=ALU.mult)
            th = s2.tile([128, 128], F32, tag="gth", name="gth")[:P, :Fr]
            nc.scalar.activation(out=th, in_=u, func=AF.Tanh)
            w = s2.tile([128, 128], F32, tag="gw", name="gw")[:P, :Fr]
            nc.vector.tensor_scalar(w, th, 0.5, 0.5, op0=ALU.mult,
                                    op1=ALU.add)
            nc.vector.tensor_tensor(out=dst, in0=x, in1=w, op=ALU.mult)

        def ln(dst, src_f32, gflag, gname, bname):
            st6 = s2.tile([128, 6], F32, tag="ln_st")
            mv = s2.tile([128, 2], F32, tag="ln_mv")
            nc.vector.bn_stats(out=st6, in_=src_f32)
            nc.vector.bn_aggr(out=mv, in_=st6)
            # rsd = rsqrt(var+eps) via the f32 exponent bit trick + 2 Newton
            # steps, entirely on DVE: keeps ACT on one activation table
            # (no Ln/Sqrt -> zero ACT_TABLE_LOAD thrash)
            vpe = s2.tile([128, 1], F32, tag="ln_vpe")
            nc.vector.tensor_scalar_add(vpe, mv[:, 1:2], 1e-5)
            ib = s2.tile([128, 1], I32, tag="ln_ib")
            nc.vector.tensor_scalar(ib, vpe.bitcast(I32), 1, None,
                                    op0=ALU.logical_shift_right)
            nib = s2.tile([128, 1], I32, tag="ln_nib")
            nc.vector.tensor_scalar(nib, ib, -1, None, op0=ALU.bitwise_xor)
            y0 = s2.tile([128, 1], F32, tag="ln_y0")
            # 0x5f3759df - ib == (0x5f3759df+1) + ~ib
            nc.vector.tensor_scalar(y0.bitcast(I32), nib, 0x5f3759e0, None,
                                    op0=ALU.add)
            u0 = s2.tile([128, 1], F32, tag="ln_u0")
            nc.vector.scalar_tensor_tensor(out=u0, in0=y0,
                                           scalar=vpe[:, 0:1], in1=y0,
                                           op0=ALU.mult, op1=ALU.mult)
            w0 = s2.tile([128, 1], F32, tag="ln_w0")
            nc.vector.tensor_scalar(w0, u0, -0.5, 1.5, op0=ALU.mult,
                                    op1=ALU.add)
            y1 = s2.tile([128, 1], F32, tag="ln_y1")
            nc.vector.tensor_tensor(out=y1, in0=y0, in1=w0, op=ALU.mult)
            u1 = s2.tile([128, 1], F32, tag="ln_u1")
            nc.vector.scalar_tensor_tensor(out=u1, in0=y1,
                                           scalar=vpe[:, 0:1], in1=y1,
                                           op0=ALU.mult, op1=ALU.mult)
            w1 = s2.tile([128, 1], F32, tag="ln_w1")
            nc.vector.tensor_scalar(w1, u1, -0.5, 1.5, op0=ALU.mult,
                                    op1=ALU.add)
            rsd = s2.tile([128, 1], F32, tag="ln_rsd")
            nc.vector.tensor_tensor(out=rsd, in0=y1, in1=w1, op=ALU.mult)
            if gflag:
                xn = s2.tile([128, 64], F32, tag="ln_xn")
                nc.vector.tensor_scalar(xn, src_f32, mv[:, 0:1], rsd[:, 0:1],
                                        op0=ALU.subtract, op1=ALU.mult)
                xg = s2.tile([128, 64], F32, tag="ln_xg")
                nc.vector.tensor_tensor(out=xg, in0=xn, in1=sb[gname],
                                        op=ALU.mult)
                nc.vector.tensor_tensor(out=dst, in0=xg, in1=sb[bname],
                                        op=ALU.add)
            else:
                nc.vector.tensor_scalar(dst, src_f32, mv[:, 0:1], rsd[:, 0:1],
                                        op0=ALU.subtract, op1=ALU.mult)

        def s2_preamble(c):
            # per-chunk preamble: Pbar, pos-mlp (hoisted to run right after
            # the chunk's G extraction so it is off the pair's tail chain)
            tp, o64 = c // 2, 64 * (c % 2)
            Gc = G[c]
            rS = s2.tile([128, 1], F32, tag="rS")
            nc.vector.reciprocal(out=rS, in_=Gc[:, 66:67])
            nc.scalar.mul(Pbarp[tp][:, o64:o64 + 64],
                          Gc[:, 0:64], rS[:, 0:1])
            cc2 = s2.tile([128, 2], F32, tag="cc2")
            nc.vector.tensor_scalar_mul(cc2, Gc[:, 64:66], rS[:, 0:1])
            t1 = s2.tile([128, 64], F32, tag="t1")
            nc.vector.tensor_scalar_mul(t1, sb["w1x_b"], cc2[:, 0:1])
            h1 = s2.tile([128, 64], F32, tag="h1")
            nc.vector.scalar_tensor_tensor(out=h1, in0=sb["w1y_b"],
                                           scalar=cc2[:, 1:2], in1=t1,
                                           op0=ALU.mult, op1=ALU.add)
            if flags["posb1"]:
                nc.vector.tensor_tensor(out=h1, in0=h1,
                                        in1=sb["posb1_b"], op=ALU.add)
            gelu(h1gp[tp][:, o64:o64 + 64], h1)

        def stage2_pair(tp):
            PbarT = s2.tile([128, 128], BF16, tag="PbarT", bufs=2)
            h1gT = s2.tile([128, 128], BF16, tag="h1gT", bufs=2)
            xbar(PbarT, Pbarp[tp])
            xbar(h1gT, h1gp[tp])
            for c2 in range(2 * tp, min(2 * tp + 2, NCHUNK)):
                o2 = 64 * (c2 % 2)
                X0 = s2p.tile([128, 64], F32, tag="ps")
                nc.tensor.matmul(X0, PbarT[o2:o2 + 64, :],
                                 sb["WvpixR"][o2:o2 + 64, :],
                                 start=True, stop=False)
                nc.tensor.matmul(X0, h1gT[o2:o2 + 64, :],
                                 sb["posw2R"][o2:o2 + 64, :],
                                 start=False, stop=True)
                if flags["bvb2"]:
                    nc.vector.tensor_tensor(out=Xpair[tp][:, o2:o2 + 64],
                                            in0=X0, in1=sb["bvb2_b"],
                                            op=ALU.add)
                else:
                    nc.scalar.copy(out=Xpair[tp][:, o2:o2 + 64], in_=X0)
            XT = s2.tile([128, 128], BF16, tag="XT", bufs=2)
            xbar(XT, Xpair[tp])
            for c2 in range(2 * tp, min(2 * tp + 2, NCHUNK)):
                o2 = 64 * (c2 % 2)
                qTp = s2p.tile([128, 128], F32, tag="ps")
                nc.tensor.matmul(qTp, sb["WqR"][o2:o2 + 64, :],
                                 XT[o2:o2 + 64, :], start=True, stop=True)
                nc.scalar.activation(out=qT_sb[:, 128 * c2:128 * (c2 + 1)],
                                     in_=qTp, func=AF.Identity,
                                     bias=sb["bq_pad"][:, 0:1])
                kTp = s2p.tile([128, 128], F32, tag="ps")
                nc.tensor.matmul(kTp, sb["WkR"][o2:o2 + 64, :],
                                 XT[o2:o2 + 64, :], start=True, stop=True)
                nc.scalar.activation(out=kT_sb[:, 128 * c2:128 * (c2 + 1)],
                                     in_=kTp, func=AF.Identity,
                                     bias=sb["bk_pad"][:, 0:1])
                Vp = s2p.tile([128, 64], F32, tag="ps")
                nc.tensor.matmul(Vp, XT[o2:o2 + 64, :],
                                 sb["WvaR"][o2:o2 + 64, :],
                                 start=True, stop=True)
                if flags["bv_attn"]:
                    nc.vector.tensor_tensor(
                        out=V_sb[:, 64 * c2:64 * (c2 + 1)], in0=Vp,
                        in1=sb["bva_b"], op=ALU.add)
                else:
                    nc.scalar.copy(out=V_sb[:, 64 * c2:64 * (c2 + 1)],
                                   in_=Vp)
            # attention per chunk: one full-tile Exp per head + block-diag
            # mask multiply (off-diag exp'd values are zeroed by bd128)
            for c2 in range(2 * tp, min(2 * tp + 2, NCHUNK)):
                o2 = 64 * (c2 % 2)
                SA = s2p.tile([128, 68], F32, tag="ps", name="SA")
                for h in range(NH):
                    SC = s2p.tile([128, 128], F32, tag="ps", name="SC")
                    nc.tensor.matmul(
                        SC,
                        qT_sb[32 * h:32 * h + DH, 128 * c2:128 * (c2 + 1)],
                        kT_sb[32 * h:32 * h + DH, 128 * c2:128 * (c2 + 1)],
                        start=True, stop=True, tile_position=(32 * h, 0))
                    etf = e2p.tile([128, 128], BF16, tag="etf")
                    nc.scalar.activation(out=etf, in_=SC, func=AF.Exp)
                    E2 = e2p.tile([128, 128], BF16, tag="E2")
                    nc.vector.tensor_tensor(out=E2, in0=etf,
                                            in1=sb["bd128"], op=ALU.mult)
                    nc.tensor.matmul(
                        SA[:, 16 * h:16 * (h + 1)], E2,
                        V_sb[:, 64 * c2 + 16 * h:64 * c2 + 16 * (h + 1)],
                        start=(h == 0), stop=False)
                    nc.tensor.matmul(SA[:, 64 + h:65 + h], E2,
                                     ones_bf[:, 0:1], start=False,
                                     stop=(h == NH - 1))
                rR = s2.tile([128, 4], F32, tag="rR")
                nc.vector.reciprocal(out=rR, in_=SA[:, 64:68])
                rRb = bass.AP(tensor=rR.tensor, offset=rR.offset,
                              ap=[rR.ap[0], rR.ap[1], [0, 16]])
                nc.vector.tensor_tensor(
                    out=saNp[tp][:, o2:o2 + 64].rearrange(
                        "p (h k) -> p h k", k=16),
                    in0=SA[:, 0:64].rearrange("p (h k) -> p h k", k=16),
                    in1=rRb, op=ALU.mult)
            saNT = s2.tile([128, 128], BF16, tag="saNT", bufs=2)
            xbar(saNT, saNp[tp])
            for c2 in range(2 * tp, min(2 * tp + 2, NCHUNK)):
                o2 = 64 * (c2 % 2)
                x1ps = s2p.tile([128, 64], F32, tag="ps")
                nc.tensor.matmul(x1ps, saNT[o2:o2 + 64, :],
                                 sb["WoR"][o2:o2 + 64, :],
                                 start=True, stop=True)
                x1s = s2.tile([128, 64], F32, tag="x1s")
                nc.vector.tensor_tensor(out=x1s, in0=x1ps,
                                        in1=Xpair[tp][:, o2:o2 + 64],
                                        op=ALU.add)
                if flags["bo"]:
                    nc.vector.tensor_tensor(out=x1s, in0=x1s,
                                            in1=sb["bo_b"], op=ALU.add)
                ln(x1p_t[tp][:, o2:o2 + 64], x1s, flags["ln1"],
                   "ln1g_b", "ln1b_b")
            x1T = s2.tile([128, 128], BF16, tag="x1T", bufs=2)
            xbar(x1T, x1p_t[tp])
            for c2 in range(2 * tp, min(2 * tp + 2, NCHUNK)):
                o2 = 64 * (c2 % 2)
                f1a = s2p.tile([128, 128], F32, tag="ps")
                nc.tensor.matmul(f1a, sb["W1aR"][o2:o2 + 64, :],
                                 x1T[o2:o2 + 64, :], start=True, stop=True)
                fga = s2.tile([128, 128], BF16, tag="fga")
                gelu(fga, f1a, bias=sb["b1a"][:, 0:1])
                f1b = s2p.tile([128, 128], F32, tag="ps")
                nc.tensor.matmul(f1b, sb["W1bR"][o2:o2 + 64, :],
                                 x1T[o2:o2 + 64, :], start=True, stop=True)
                fgb = s2.tile([128, 128], BF16, tag="fgb")
                gelu(fgb, f1b, bias=sb["b1b"][:, 0:1])
                f2 = s2p.tile([128, 64], F32, tag="ps")
                nc.tensor.matmul(f2, fga, sb["W2a"], start=True, stop=False)
                nc.tensor.matmul(f2, fgb, sb["W2b"], start=False, stop=True)
                x2s = s2.tile([128, 64], F32, tag="x2s")
                nc.vector.tensor_tensor(out=x2s, in0=f2,
                                        in1=x1p_t[tp][:, o2:o2 + 64],
                                        op=ALU.add)
                if flags["b2ffn"]:
                    nc.vector.tensor_tensor(out=x2s, in0=x2s,
                                            in1=sb["b2f_b"], op=ALU.add)
                ln(x2p_t[tp][:, o2:o2 + 64], x2s, flags["ln2"],
                   "ln2g_b", "ln2b_b")
            x2T = s2.tile([128, 128], BF16, tag="x2T", bufs=2)
            xbar(x2T, x2p_t[tp])
            for c2 in range(2 * tp, min(2 * tp + 2, NCHUNK)):
                o2 = 64 * (c2 % 2)
                KTp = s2p.tile([64, 128], F32, tag="ps")
                nc.tensor.matmul(KTp, sb["WkclsR"][o2:o2 + 64, :],
                                 x2T[o2:o2 + 64, :], start=True, stop=True)
                nc.scalar.activation(
                    out=KclsT_sb[:, 128 * c2:128 * (c2 + 1)], in_=KTp,
                    func=AF.Identity, bias=sb["bkcls"][0:64, 0:1])
                wvtp = s2p.tile([1, 128], F32, tag="ps")
                nc.tensor.matmul(wvtp, sb["wvecR"][o2:o2 + 64, 0:1],
                                 x2T[o2:o2 + 64, :], start=True, stop=True)
                nc.scalar.activation(out=wvT_sb[:, 128 * c2:128 * (c2 + 1)],
                                     in_=wvtp, func=AF.Copy,
                                     bias=flags["cvh"])
            for c2 in range(2 * tp, min(2 * tp + 2, NCHUNK)):
                SCc = s2p.tile([C, 128], F32, tag="ps")
                nc.tensor.matmul(SCc, sb["CqT"][0:64, :],
                                 KclsT_sb[:, 128 * c2:128 * (c2 + 1)],
                                 start=True, stop=True)
                E2c = s2.tile([C, 128], F32, tag="E2c")
                nc.scalar.activation(out=E2c, in_=SCc, func=AF.Exp)
                wv7 = s2p.tile([C, 128], F32, tag="ps")
                nc.tensor.matmul(wv7, ones7,
                                 wvT_sb[:, 128 * c2:128 * (c2 + 1)],
                                 start=True, stop=True)
                prod = s2.tile([C, 128], F32, tag="prod")
                nc.vector.tensor_tensor(out=prod, in0=E2c, in1=wv7,
                                        op=ALU.mult)
                num = s2.tile([C, 8], F32, tag="num")
                nc.vector.reduce_sum(
                    out=num, in_=prod.rearrange("c (b k) -> c b k", k=16),
                    axis=mybir.AxisListType.X)
                den = s2.tile([C, 8], F32, tag="den")
                nc.vector.reduce_sum(
                    out=den, in_=E2c.rearrange("c (b k) -> c b k", k=16),
                    axis=mybir.AxisListType.X)
                rden = s2.tile([C, 8], F32, tag="rden")
                nc.vector.reciprocal(out=rden, in_=den)
                oc = s2.tile([C, 8], F32, tag="oc")
                nc.vector.tensor_tensor(out=oc, in0=num, in1=rden,
                                        op=ALU.mult)
                nc.vector.tensor_scalar_add(
                    OUT_sb[:, 8 * c2:8 * (c2 + 1)], oc, flags["bh"])

        # ================= stage 1 + interleaved stage 2 =================
        # 16-batch load-groups (2 loads + 2 xbars total): the deadlock guard
        # serializes SWDGE DMAs against DMA-transposes one unit at a time,
        # so fewer/bigger units minimize the serialization hops. Chunk
        # structure (8 batches x 16 slots per 128 partitions) is unchanged.
        GB = 16
        NLG = NG // 2
        hqs = []
        for lg in range(NLG):
            hq = hbp.tile([128, GB, NT * 64], BF16, tag="hq")
            nc.gpsimd.dma_start(
                out=hq,
                in_=hp.ap()[GB * lg:GB * (lg + 1)].rearrange(
                    "b (p q) d -> p b (q d)", p=128))
            hqs.append(hq)
        for lg in range(NLG):
            hq = hqs[lg]
            xt = xtp.tile([128, GB * NT // 2, 128], BF16, tag="xt")
            # tiled xbar transpose: chunk 9b+c = [2q*64d, 128 tok]
            nc.sync.dma_start(out=xt, in_=hq, transpose=True)
            if mk_d is not None:
                mk = smp.tile([128, GB, NT], I32, tag="mk")
                nc.sync.dma_start(
                    out=mk,
                    in_=mk_d.ap()[GB * lg:GB * (lg + 1)].rearrange(
                        "b (p q) -> p b q", p=128))
                mkf = smp.tile([128, GB, NT], F32, tag="mkf")
                nc.vector.tensor_copy(out=mkf, in_=mk)
            P8s = [ppp.tile([128, 512], F32, tag="P8", name=f"P8_{lg}_{ch}")
                   for ch in range(2)]
            Pps = [pop.tile([128, 3], F32, tag="Pp", name=f"Pp_{lg}_{ch}")
                   for ch in range(2)]
            for c in range(NCT):
                LG = lgp.tile([128, 32 * GB], F32, tag="LG")
                for b in range(GB):
                    nc.tensor.matmul(LG[:, 32 * b:32 * (b + 1)],
                                     xt[:, NCT * b + c, :], sb["AT2"],
                                     start=True, stop=True)
                E8 = smp.tile([128, 32 * GB], BF16, tag="E8")
                nc.scalar.activation(out=E8, in_=LG, func=AF.Exp)
                if flags["c0"]:
                    ec = sb["expc0_b"]
                    ecb = bass.AP(tensor=ec.tensor, offset=ec.offset,
                                  ap=[ec.ap[0], [0, 2 * GB], ec.ap[1]])
                    nc.vector.tensor_tensor(
                        out=E8.rearrange("p (g k) -> p g k", k=K),
                        in0=E8.rearrange("p (g k) -> p g k", k=K),
                        in1=ecb, op=ALU.mult)
                sig = smp.tile([128, 2 * GB], F32, tag="sig")
                nc.vector.tensor_reduce(
                    out=sig, in_=E8.rearrange("p (g k) -> p g k", k=K),
                    op=ALU.add, axis=mybir.AxisListType.X)
                rsig = smp.tile([128, 2 * GB], F32, tag="rsig")
                nc.vector.reciprocal(out=rsig, in_=sig)
                if mk_d is not None:
                    nc.vector.tensor_tensor(
                        out=rsig.rearrange("p (b q) -> p b q", q=2),
                        in0=rsig.rearrange("p (b q) -> p b q", q=2),
                        in1=mkf[:, :, 2 * c:2 * c + 2], op=ALU.mult)
                # pm stored parity-major [p, qp, b, k] so each parity
                # half-slice is a contiguous [128, 128] stationary operand
                pm8 = smp.tile([128, 2, GB, K], BF16, tag="pm8")
                rsv = rsig.rearrange("p (b q) -> p b q", q=2)
                rb = bass.AP(tensor=rsv.tensor, offset=rsv.offset,
                             ap=[rsv.ap[0], rsv.ap[1], rsv.ap[2], [0, K]])
                nc.vector.tensor_tensor(
                    out=pm8.rearrange("p q b k -> p b q k"),
                    in0=E8.rearrange("p (b q k) -> p b q k", q=2, k=K),
                    in1=rb, op=ALU.mult)
                for ch in range(2):
                    for qp in range(2):
                        st = (c == 0 and qp == 0)
                        sp = (c == NCT - 1 and qp == 1)
                        nc.tensor.matmul(
                            P8s[ch], pm8[:, qp, 8 * ch:8 * (ch + 1), :],
                            hq[:, 8 * ch:8 * (ch + 1),
                               64 * (2 * c + qp):64 * (2 * c + qp + 1)],
                            start=st, stop=sp)
                        nc.tensor.matmul(Pps[ch],
                                         pm8[:, qp, 8 * ch:8 * (ch + 1), :],
                                         sb["posE"][:, 2 * c + qp, :],
                                         start=st, stop=sp)
            # G extraction per chunk: masked accumulate over column blocks
            for ch in range(2):
                Gg = G[2 * lg + ch]
                P8 = P8s[ch]
                nc.vector.tensor_scalar_mul(Gg[:, 0:64], P8[:, 0:64],
                                            sb["mask8"][:, 0:1])
                for b in range(1, 8):
                    nc.vector.scalar_tensor_tensor(
                        out=Gg[:, 0:64], in0=P8[:, 64 * b:64 * (b + 1)],
                        scalar=sb["mask8"][:, b:b + 1], in1=Gg[:, 0:64],
                        op0=ALU.mult, op1=ALU.add)
                nc.vector.tensor_copy(out=Gg[:, 64:67], in_=Pps[ch])
                s2_preamble(2 * lg + ch)
            stage2_pair(lg)
        nc.sync.dma_start(out=out_d.ap().rearrange("b c -> c b"), in_=OUT_sb)


_CACHE = {}
TRACE = False          # test harness can set kernel.TRACE = True
LAST_RESULT = None     # BassKernelResults of the last kernel() call


def _get_program(nbatch, n, use_mask, flags, blob_cols, meta):
    key = (nbatch, n, use_mask, tuple(sorted(
        (k, v) for k, v in flags.items() if isinstance(v, bool))))
    if key not in _CACHE:
        _CACHE[key] = build(nbatch, n, use_mask, flags, blob_cols, meta)
    return _CACHE[key]


def kernel(**inputs):
    from concourse.bass_utils import run_bass_kernel_spmd

    h_pixel = np.ascontiguousarray(np.asarray(inputs["h_pixel"],
                                              dtype=np.float32))
    node_mask = np.ascontiguousarray(np.asarray(inputs["node_mask"],
                                                dtype=np.int32))
    b, n, d = h_pixel.shape
    params = {k: v for k, v in inputs.items()
              if k not in ("h_pixel", "node_mask")}
    blobs, meta, flags = host_prep(params, n=n)
    blob_cols = {BF16: blobs[BF16].shape[1], F32: blobs[F32].shape[1]}
    use_mask = bool(not np.all(node_mask == 1))
    nbatch = b // NCORES
    nc = _get_program(nbatch, n, use_mask, flags, blob_cols, meta)

    in_maps = []
    for core in range(NCORES):
        m = {"hp": h_pixel[core * nbatch:(core + 1) * nbatch],
             "blob_bf": blobs[BF16], "blob_f32": blobs[F32]}
        if use_mask:
            m["mask"] = node_mask[core * nbatch:(core + 1) * nbatch]
        in_maps.append(m)
    kwargs = {}
    if TRACE:
        kwargs["trace"] = True
    res = run_bass_kernel_spmd(nc, in_maps, core_ids=list(range(NCORES)),
                               **kwargs)
    global LAST_RESULT
    LAST_RESULT = res
    out = np.concatenate([r["out"] for r in res.results], axis=0)
    return out.astype(np.float32)


if __name__ == "__main__":
    sys.path.insert(0, "/root/problem")
    import reference
    inputs = {k: np.asarray(v) for k, v in reference.setup_inputs().items()}
    got = kernel(**inputs)
    print("out shape", got.shape)


# revision 24
# speedup vs baseline: 1.0038x; 1.0038x over previous
"""Trainium2 Bass kernel for nn_D6BPixelMotifBranch (pooling / memory-bound).

Data-parallel over batch across 8 NeuronCores; 4 groups of 8 batches per core.
Stage 1 (per group):
  - one contiguous-descriptor SWDGE cast-DMA (f32->bf16) per group with
    token permutation token = p*NT + q  (partition p holds NT consecutive
    tokens), giving 4.6KB descriptor runs (single big DMAs also minimize
    the serialization hops of the DMA-transpose deadlock guard);
  - one DMA-xbar tiled transpose per group produces hT chunks
    [2q*64d, 128 tok] directly (no PE transposes, no PSUM->SBUF copies);
  - per chunk-tile: 8 logits matmuls (lhsT = hT chunk, rhs = block-diag AT2),
    batched softmax over slots in [tok, 8b*2q*16k] layout, parity-sliced
    full-width pooling matmuls accumulate G in [8b*16k, 4b*64d] PSUM layout;
  - G extraction = pure column-select copies (no partition shifts, no DMA).
Stage 2 (per pair of groups, interleaved): post-norm transformer block +
class cross-attention, batched 8 batches x 16 slots per 128 partitions.
LayerNorm rsqrt runs on DVE via the f32 exponent bit trick (no Ln/Sqrt)
and gelu uses a Square/Tanh decomposition, so the scalar engine stays on a
single activation table (one ACT_TABLE_LOAD for the whole kernel).
"""

import sys
for _p in ("/opt/trn_rl_repo", "/root/.axon_site/_ro/trn_rl_repo"):
    if _p not in sys.path:
        sys.path.append(_p)

import numpy as np
import ml_dtypes

import concourse.bacc as bacc
import concourse.tile as tile
from concourse import mybir

BF16 = mybir.dt.bfloat16
F32 = mybir.dt.float32
I32 = mybir.dt.int32
AF = mybir.ActivationFunctionType
ALU = mybir.AluOpType

# problem dims
B, N, D, K, C, NH, HGT, WID = 256, 2304, 64, 16, 7, 4, 48, 48
DH = D // NH
FF = 4 * D
NCORES = 8

_bf = lambda x: np.ascontiguousarray(x.astype(ml_dtypes.bfloat16))
_f32 = lambda x: np.ascontiguousarray(x.astype(np.float32))


def _positions(n):
    ys = np.linspace(0.0, 1.0, HGT, dtype=np.float64)
    xs = np.linspace(0.0, 1.0, WID, dtype=np.float64)
    yy, xx = np.meshgrid(ys, xs, indexing="ij")
    pos = np.stack([xx.reshape(-1), yy.reshape(-1)], axis=-1)  # [N,2]
    return pos[:n].astype(np.float64)


class BlobPacker:
    """Pack named [128, w] host arrays into one blob per dtype."""

    def __init__(self):
        self.cols = {BF16: 0, F32: 0}
        self.items = []          # (name, dtype, off, shape, array)

    def add(self, name, arr, dt):
        a = np.asarray(arr)
        if a.ndim == 1:
            a = a[None, :]
        rows, w = a.shape[0], int(np.prod(a.shape[1:]))
        flat = np.zeros((128, w), np.float64)
        flat[:rows, :] = a.reshape(rows, w)
        off = self.cols[dt]
        self.cols[dt] = off + w
        self.items.append((name, dt, off, tuple(a.shape), flat))

    def finalize(self):
        blobs = {}
        for dt, conv in ((BF16, _bf), (F32, _f32)):
            w = max(self.cols[dt], 1)
            blob = np.zeros((128, w), np.float64)
            for name, d, off, shape, flat in self.items:
                if d is dt:
                    blob[:, off:off + flat.shape[1]] = flat
            blobs[dt] = conv(blob)
        meta = {name: (dt, off, shape) for name, dt, off, shape, _ in self.items}
        return blobs, meta


def host_prep(params, n=N):
    """Precompute all constant tensors, packed into two blobs."""
    p = {k: np.asarray(v, dtype=np.float64) for k, v in params.items()}
    scale = 1.0 / np.sqrt(np.float64(D))
    q = p["part_queries"]
    q = q / np.maximum(np.linalg.norm(q, axis=-1, keepdims=True), 1e-6)
    NT = n // 128

    pk = BlobPacker()
    flags = {}

    # slot logits: h @ AT + c0; block-diag [128, 32] for 2q (or 2b) pairing
    AT = (p["Wk_pix"] @ q.T) * scale                     # [64, 16]
    c0 = (p["bk_pix"] @ q.T) * scale                     # [16]
    AT2 = np.zeros((128, 32))
    AT2[0:64, 0:16] = AT
    AT2[64:128, 16:32] = AT
    pk.add("AT2", AT2, BF16)
    flags["c0"] = bool(np.any(c0 != 0.0))
    pk.add("expc0_b", np.tile(np.exp(c0)[None, :], (128, 1)), F32)

    # positions (x, y, 1), token-permuted: token = p*NT + q
    posE = np.concatenate([_positions(n), np.ones((n, 1))], -1)  # [n, 3]
    pk.add("posE", posE.reshape(128, NT, 3), BF16)

    # post-pool projection of raw pooled h
    pk.add("WvpixR", np.vstack([p["Wv_pix"], p["Wv_pix"]]), BF16)
    bvb2 = p["bv_pix"] + p["pos_b2"]
    flags["bvb2"] = bool(np.any(bvb2 != 0.0))
    pk.add("bvb2_b", np.tile(bvb2[None, :], (128, 1)), F32)

    # pos mlp
    pk.add("w1x_b", np.tile(p["pos_w1"][0][None, :], (128, 1)), F32)
    pk.add("w1y_b", np.tile(p["pos_w1"][1][None, :], (128, 1)), F32)
    flags["posb1"] = bool(np.any(p["pos_b1"] != 0.0))
    pk.add("posb1_b", np.tile(p["pos_b1"][None, :], (128, 1)), F32)
    pk.add("posw2R", np.vstack([p["pos_w2"], p["pos_w2"]]), BF16)

    # attention qkv with head padding: head h at partitions 32h..32h+16
    attn_scale = 1.0 / np.sqrt(np.float64(DH))
    Wq = p["Wqkv"][:, 0:D] * attn_scale
    Wk = p["Wqkv"][:, D:2 * D]
    Wv = p["Wqkv"][:, 2 * D:3 * D]
    bq = p["bqkv"][0:D] * attn_scale
    bk = p["bqkv"][D:2 * D]
    bv = p["bqkv"][2 * D:3 * D]
    Wq_pad = np.zeros((D, 128))
    Wk_pad = np.zeros((D, 128))
    bq_pad = np.zeros(128)
    bk_pad = np.zeros(128)
    for h in range(NH):
        Wq_pad[:, 32 * h:32 * h + DH] = Wq[:, DH * h:DH * (h + 1)]
        Wk_pad[:, 32 * h:32 * h + DH] = Wk[:, DH * h:DH * (h + 1)]
        bq_pad[32 * h:32 * h + DH] = bq[DH * h:DH * (h + 1)]
        bk_pad[32 * h:32 * h + DH] = bk[DH * h:DH * (h + 1)]
    pk.add("WqR", np.vstack([Wq_pad, Wq_pad]), BF16)     # [128, 128]
    pk.add("WkR", np.vstack([Wk_pad, Wk_pad]), BF16)
    pk.add("bq_pad", bq_pad[:, None], F32)               # [128, 1]
    pk.add("bk_pad", bk_pad[:, None], F32)
    pk.add("WvaR", np.vstack([Wv, Wv]), BF16)            # [128, 64]
    flags["bv_attn"] = bool(np.any(bv != 0.0))
    pk.add("bva_b", np.tile(bv[None, :], (128, 1)), F32)

    pk.add("WoR", np.vstack([p["Wo"], p["Wo"]]), BF16)
    flags["bo"] = bool(np.any(p["bo"] != 0.0))
    pk.add("bo_b", np.tile(p["bo"][None, :], (128, 1)), F32)

    flags["ln1"] = bool(np.any(p["ln1_g"] != 1.0) or np.any(p["ln1_b"] != 0.0))
    pk.add("ln1g_b", np.tile(p["ln1_g"][None, :], (128, 1)), F32)
    pk.add("ln1b_b", np.tile(p["ln1_b"][None, :], (128, 1)), F32)
    flags["ln2"] = bool(np.any(p["ln2_g"] != 1.0) or np.any(p["ln2_b"] != 0.0))
    pk.add("ln2g_b", np.tile(p["ln2_g"][None, :], (128, 1)), F32)
    pk.add("ln2b_b", np.tile(p["ln2_b"][None, :], (128, 1)), F32)

    # ffn
    W1 = p["ffn_w1"]                                     # [64, 256]
    pk.add("W1aR", np.vstack([W1[:, 0:128], W1[:, 0:128]]), BF16)
    pk.add("W1bR", np.vstack([W1[:, 128:256], W1[:, 128:256]]), BF16)
    pk.add("b1a", p["ffn_b1"][0:128, None], F32)         # [128, 1]
    pk.add("b1b", p["ffn_b1"][128:256, None], F32)
    pk.add("W2a", p["ffn_w2"][0:128, :], BF16)           # [128, 64]
    pk.add("W2b", p["ffn_w2"][128:256, :], BF16)
    flags["b2ffn"] = bool(np.any(p["ffn_b2"] != 0.0))
    pk.add("b2f_b", np.tile(p["ffn_b2"][None, :], (128, 1)), F32)

    # class cross attention
    pk.add("WkclsR", np.vstack([p["Wk_cls"], p["Wk_cls"]]), BF16)
    pk.add("bkcls", np.concatenate([p["bk_cls"], np.zeros(64)])[:, None], F32)
    pk.add("CqT", (p["class_queries"] * scale).T, BF16)  # [64, 7]
    wvec = p["Wv_cls"] @ p["Wh"]                         # [64, 1]
    pk.add("wvecR", np.vstack([wvec, wvec]), BF16)       # [128, 1]
    flags["cvh"] = float(p["bv_cls"] @ p["Wh"][:, 0])
    flags["bh"] = float(p["bh"][0])

    pk.add("ident", np.eye(128), BF16)
    # full block-diag mask: 1 iff same 16-slot (batch) block
    bd128 = np.kron(np.eye(8), np.ones((16, 16)))
    pk.add("bd128", bd128, BF16)
    # G-extraction partition masks: mask8[p, b] = 1 iff p//16 == b
    m8 = np.zeros((128, 8))
    for pp in range(128):
        m8[pp, pp // 16] = 1.0
    pk.add("mask8", m8, F32)
    # block-diag mask for zeroing cross-batch attention scores
    bd = np.zeros((128, 32))
    for pp in range(128):
        half = (pp % 32) // 16
        bd[pp, 16 * half:16 * (half + 1)] = 1.0
    pk.add("bdiag", bd, BF16)

    blobs, meta = pk.finalize()
    return blobs, meta, flags


def build(nbatch, n, use_mask, flags, blob_cols, meta):
    """Build the per-core Bass program. nbatch must be a multiple of 16."""
    NT = n // 128
    assert nbatch % 16 == 0 and n % 256 == 0 and NT % 2 == 0
    NG = nbatch // 8
    NCT = NT // 2

    nc = bacc.Bacc("TRN2", debug=False)

    hp = nc.dram_tensor("hp", [nbatch, n, D], F32, kind="ExternalInput")
    if use_mask:
        mk_d = nc.dram_tensor("mask", [nbatch, n], I32, kind="ExternalInput")
    blob_bf_d = nc.dram_tensor("blob_bf", [128, blob_cols[BF16]], BF16,
                               kind="ExternalInput")
    blob_f32_d = nc.dram_tensor("blob_f32", [128, blob_cols[F32]], F32,
                                kind="ExternalInput")
    out_d = nc.dram_tensor("out", [nbatch, C], F32, kind="ExternalOutput")

    with tile.TileContext(nc) as tc:
        _build_body(nc, tc, hp, mk_d if use_mask else None,
                    (blob_bf_d, blob_f32_d), out_d,
                    nbatch, n, NT, NG, NCT, flags, meta)
    nc.compile()
    return nc


def _build_body(nc, tc, hp, mk_d, blob_drams, out_d, nbatch, n, NT, NG, NCT,
                flags, meta):
    from contextlib import ExitStack
    import concourse.bass as bass
    ctx = ExitStack()
    with ctx:
        cst = ctx.enter_context(tc.tile_pool(name="cst", bufs=1))
        # ---- constants: two blob DMAs, then views ----
        blob_bf_d, blob_f32_d = blob_drams
        blob_bf = cst.tile(list(blob_bf_d.shape), BF16, name="blob_bf")
        blob_f32 = cst.tile(list(blob_f32_d.shape), F32, name="blob_f32")
        nc.sync.dma_start(out=blob_bf, in_=blob_bf_d.ap())
        nc.sync.dma_start(out=blob_f32, in_=blob_f32_d.ap())
        sb = {}
        for name, (dt, off, shape) in meta.items():
            blob = blob_bf if dt is BF16 else blob_f32
            w = int(np.prod(shape[1:]))
            v = blob[:shape[0], off:off + w]
            if len(shape) == 3:
                v = v.rearrange("p (a b) -> p a b", b=shape[2])
            sb[name] = v
        ones_bf = cst.tile([128, 1], BF16)
        nc.vector.memset(ones_bf, 1.0)
        eps_t = cst.tile([128, 1], F32)
        nc.vector.memset(eps_t, 1e-5)
        half_t = cst.tile([128, 1], F32)
        nc.vector.memset(half_t, 0.5)
        ones7 = cst.tile([1, C], BF16)
        nc.vector.memset(ones7, 1.0)
        # pooled stats per group: [128 (8b x 16k), 64 P | 2 C | 1 S]
        G = [cst.tile([128, 67], F32, name=f"G{g}") for g in range(NG)]
        OUT_sb = cst.tile([C, nbatch], F32)

        # ---- pools ----
        hbp = ctx.enter_context(tc.tile_pool(name="hbp", bufs=2))
        xtp = ctx.enter_context(tc.tile_pool(name="xtp", bufs=2))
        smp = ctx.enter_context(tc.tile_pool(name="smp", bufs=4))
        lgp = ctx.enter_context(tc.tile_pool(name="lgp", bufs=2, space="PSUM"))
        ppp = ctx.enter_context(tc.tile_pool(name="ppp", bufs=2, space="PSUM"))
        pop = ctx.enter_context(tc.tile_pool(name="pop", bufs=2, space="PSUM"))

        s2 = ctx.enter_context(tc.tile_pool(name="s2", bufs=2))
        s2p = ctx.enter_context(tc.tile_pool(name="s2p", bufs=2, space="PSUM"))
        e2p = ctx.enter_context(tc.tile_pool(name="e2p", bufs=4))

        # persistent stage-2 tensors
        NCHUNK = NG
        qT_sb = cst.tile([128, 128 * NCHUNK], BF16)
        kT_sb = cst.tile([128, 128 * NCHUNK], BF16)
        KclsT_sb = cst.tile([64, 128 * NCHUNK], BF16)
        wvT_sb = cst.tile([1, 128 * NCHUNK], BF16)
        V_sb = cst.tile([128, 64 * NCHUNK], BF16)
        NXP = (NCHUNK + 1) // 2
        Xpair = [cst.tile([128, 128], BF16, name=f"Xp{i}") for i in range(NXP)]
        Pbarp = [cst.tile([128, 128], BF16, name=f"Pb{i}") for i in range(NXP)]
        h1gp = [cst.tile([128, 128], BF16, name=f"h1g{i}") for i in range(NXP)]
        saNp = [cst.tile([128, 128], BF16, name=f"saN{i}") for i in range(NXP)]
        x1p_t = [cst.tile([128, 128], BF16, name=f"x1p{i}") for i in range(NXP)]
        x2p_t = [cst.tile([128, 128], BF16, name=f"x2p{i}") for i in range(NXP)]

        # ---------------- stage-2 helpers ----------------
        def xbar(dst, src):
            # PE-based [128,128] transpose (stage-2 chain-latency critical)
            ptx = s2p.tile([128, 128], F32, tag="ps", name="ptx")
            nc.tensor.matmul(ptx, src, sb["ident"], start=True, stop=True)
            nc.scalar.copy(out=dst, in_=ptx)

        def gelu(dst, src, bias=None):
            """tanh-approx gelu; ACT uses only Square/Tanh (both share the
            exp activation table -> no ACT_TABLE_LOAD thrash)."""
            P, Fr = src.shape[0], src.shape[1]
            a_c = 0.7978845608028654
            b_c = 0.044715
            if bias is not None:
                x = s2.tile([128, 128], F32, tag="gx", name="gx")[:P, :Fr]
                nc.scalar.activation(out=x, in_=src, func=AF.Identity,
                                     bias=bias)
            else:
                x = src
            sq = s2.tile([128, 128], F32, tag="gsq", name="gsq")[:P, :Fr]
            nc.scalar.activation(out=sq, in_=x, func=AF.Square)
            v = s2.tile([128, 128], F32, tag="gv", name="gv")[:P, :Fr]
            nc.vector.tensor_scalar(v, sq, a_c * b_c, a_c,
                                    op0=ALU.mult, op1=ALU.add)
            u = s2.tile([128, 128], F32, tag="gu", name="gu")[:P, :Fr]
            nc.vector.tensor_tensor(out=u, in0=x, in1=v, op=ALU.mult)
            th = s2.tile([128, 128], F32, tag="gth", name="gth")[:P, :Fr]
            nc.scalar.activation(out=th, in_=u, func=AF.Tanh)
            w = s2.tile([128, 128], F32, tag="gw", name="gw")[:P, :Fr]
            nc.vector.tensor_scalar(w, th, 0.5, 0.5, op0=ALU.mult,
                                    op1=ALU.add)
            nc.vector.tensor_tensor(out=dst, in0=x, in1=w, op=ALU.mult)

        def ln(dst, src_f32, gflag, gname, bname):
            st6 = s2.tile([128, 6], F32, tag="ln_st")
            mv = s2.tile([128, 2], F32, tag="ln_mv")
            nc.vector.bn_stats(out=st6, in_=src_f32)
            nc.vector.bn_aggr(out=mv, in_=st6)
            # rsd = rsqrt(var+eps) via the f32 exponent bit trick + 2 Newton
            # steps, entirely on DVE: keeps ACT on one activation table
            # (no Ln/Sqrt -> zero ACT_TABLE_LOAD thrash)
            vpe = s2.tile([128, 1], F32, tag="ln_vpe")
            nc.vector.tensor_scalar_add(vpe, mv[:, 1:2], 1e-5)
            ib = s2.tile([128, 1], I32, tag="ln_ib")
            nc.vector.tensor_scalar(ib, vpe.bitcast(I32), 1, None,
                                    op0=ALU.logical_shift_right)
            nib = s2.tile([128, 1], I32, tag="ln_nib")
            nc.vector.tensor_scalar(nib, ib, -1, None, op0=ALU.bitwise_xor)
            y0 = s2.tile([128, 1], F32, tag="ln_y0")
            # 0x5f3759df - ib == (0x5f3759df+1) + ~ib
            nc.vector.tensor_scalar(y0.bitcast(I32), nib, 0x5f3759e0, None,
                                    op0=ALU.add)
            u0 = s2.tile([128, 1], F32, tag="ln_u0")
            nc.vector.scalar_tensor_tensor(out=u0, in0=y0,
                                           scalar=vpe[:, 0:1], in1=y0,
                                           op0=ALU.mult, op1=ALU.mult)
            w0 = s2.tile([128, 1], F32, tag="ln_w0")
            nc.vector.tensor_scalar(w0, u0, -0.5, 1.5, op0=ALU.mult,
                                    op1=ALU.add)
            y1 = s2.tile([128, 1], F32, tag="ln_y1")
            nc.vector.tensor_tensor(out=y1, in0=y0, in1=w0, op=ALU.mult)
            u1 = s2.tile([128, 1], F32, tag="ln_u1")
            nc.vector.scalar_tensor_tensor(out=u1, in0=y1,
                                           scalar=vpe[:, 0:1], in1=y1,
                                           op0=ALU.mult, op1=ALU.mult)
            w1 = s2.tile([128, 1], F32, tag="ln_w1")
            nc.vector.tensor_scalar(w1, u1, -0.5, 1.5, op0=ALU.mult,
                                    op1=ALU.add)
            rsd = s2.tile([128, 1], F32, tag="ln_rsd")
            nc.vector.tensor_tensor(out=rsd, in0=y1, in1=w1, op=ALU.mult)
            if gflag:
                xn = s2.tile([128, 64], F32, tag="ln_xn")
                nc.vector.tensor_scalar(xn, src_f32, mv[:, 0:1], rsd[:, 0:1],
                                        op0=ALU.subtract, op1=ALU.mult)
                xg = s2.tile([128, 64], F32, tag="ln_xg")
                nc.vector.tensor_tensor(out=xg, in0=xn, in1=sb[gname],
                                        op=ALU.mult)
                nc.vector.tensor_tensor(out=dst, in0=xg, in1=sb[bname],
                                        op=ALU.add)
            else:
                nc.vector.tensor_scalar(dst, src_f32, mv[:, 0:1], rsd[:, 0:1],
                                        op0=ALU.subtract, op1=ALU.mult)

        def s2_preamble(c):
            # per-chunk preamble: Pbar, pos-mlp (hoisted to run right after
            # the chunk's G extraction so it is off the pair's tail chain)
            tp, o64 = c // 2, 64 * (c % 2)
            Gc = G[c]
            rS = s2.tile([128, 1], F32, tag="rS")
            nc.vector.reciprocal(out=rS, in_=Gc[:, 66:67])
            nc.scalar.mul(Pbarp[tp][:, o64:o64 + 64],
                          Gc[:, 0:64], rS[:, 0:1])
            cc2 = s2.tile([128, 2], F32, tag="cc2")
            nc.vector.tensor_scalar_mul(cc2, Gc[:, 64:66], rS[:, 0:1])
            t1 = s2.tile([128, 64], F32, tag="t1")
            nc.vector.tensor_scalar_mul(t1, sb["w1x_b"], cc2[:, 0:1])
            h1 = s2.tile([128, 64], F32, tag="h1")
            nc.vector.scalar_tensor_tensor(out=h1, in0=sb["w1y_b"],
                                           scalar=cc2[:, 1:2], in1=t1,
                                           op0=ALU.mult, op1=ALU.add)
            if flags["posb1"]:
                nc.vector.tensor_tensor(out=h1, in0=h1,
                                        in1=sb["posb1_b"], op=ALU.add)
            gelu(h1gp[tp][:, o64:o64 + 64], h1)

        def stage2_pair(tp):
            PbarT = s2.tile([128, 128], BF16, tag="PbarT", bufs=2)
            h1gT = s2.tile([128, 128], BF16, tag="h1gT", bufs=2)
            xbar(PbarT, Pbarp[tp])
            xbar(h1gT, h1gp[tp])
            for c2 in range(2 * tp, min(2 * tp + 2, NCHUNK)):
                o2 = 64 * (c2 % 2)
                X0 = s2p.tile([128, 64], F32, tag="ps")
                nc.tensor.matmul(X0, PbarT[o2:o2 + 64, :],
                                 sb["WvpixR"][o2:o2 + 64, :],
                                 start=True, stop=False)
                nc.tensor.matmul(X0, h1gT[o2:o2 + 64, :],
                                 sb["posw2R"][o2:o2 + 64, :],
                                 start=False, stop=True)
                if flags["bvb2"]:
                    nc.vector.tensor_tensor(out=Xpair[tp][:, o2:o2 + 64],
                                            in0=X0, in1=sb["bvb2_b"],
                                            op=ALU.add)
                else:
                    nc.scalar.copy(out=Xpair[tp][:, o2:o2 + 64], in_=X0)
            XT = s2.tile([128, 128], BF16, tag="XT", bufs=2)
            xbar(XT, Xpair[tp])
            for c2 in range(2 * tp, min(2 * tp + 2, NCHUNK)):
                o2 = 64 * (c2 % 2)
                qTp = s2p.tile([128, 128], F32, tag="ps")
                nc.tensor.matmul(qTp, sb["WqR"][o2:o2 + 64, :],
                                 XT[o2:o2 + 64, :], start=True, stop=True)
                nc.scalar.activation(out=qT_sb[:, 128 * c2:128 * (c2 + 1)],
                                     in_=qTp, func=AF.Identity,
                                     bias=sb["bq_pad"][:, 0:1])
                kTp = s2p.tile([128, 128], F32, tag="ps")
                nc.tensor.matmul(kTp, sb["WkR"][o2:o2 + 64, :],
                                 XT[o2:o2 + 64, :], start=True, stop=True)
                nc.scalar.activation(out=kT_sb[:, 128 * c2:128 * (c2 + 1)],
                                     in_=kTp, func=AF.Identity,
                                     bias=sb["bk_pad"][:, 0:1])
                Vp = s2p.tile([128, 64], F32, tag="ps")
                nc.tensor.matmul(Vp, XT[o2:o2 + 64, :],
                                 sb["WvaR"][o2:o2 + 64, :],
                                 start=True, stop=True)
                if flags["bv_attn"]:
                    nc.vector.tensor_tensor(
                        out=V_sb[:, 64 * c2:64 * (c2 + 1)], in0=Vp,
                        in1=sb["bva_b"], op=ALU.add)
                else:
                    nc.scalar.copy(out=V_sb[:, 64 * c2:64 * (c2 + 1)],
                                   in_=Vp)
            # attention per chunk: one full-tile Exp per head + block-diag
            # mask multiply (off-diag exp'd values are zeroed by bd128)
            for c2 in range(2 * tp, min(2 * tp + 2, NCHUNK)):
                o2 = 64 * (c2 % 2)
                SA = s2p.tile([128, 68], F32, tag="ps", name="SA")
                for h in range(NH):
                    SC = s2p.tile([128, 128], F32, tag="ps", name="SC")
                    nc.tensor.matmul(
                        SC,
                        qT_sb[32 * h:32 * h + DH, 128 * c2:128 * (c2 + 1)],
                        kT_sb[32 * h:32 * h + DH, 128 * c2:128 * (c2 + 1)],
                        start=True, stop=True, tile_position=(32 * h, 0))
                    etf = e2p.tile([128, 128], BF16, tag="etf")
                    nc.scalar.activation(out=etf, in_=SC, func=AF.Exp)
                    E2 = e2p.tile([128, 128], BF16, tag="E2")
                    nc.vector.tensor_tensor(out=E2, in0=etf,
                                            in1=sb["bd128"], op=ALU.mult)
                    nc.tensor.matmul(
                        SA[:, 16 * h:16 * (h + 1)], E2,
                        V_sb[:, 64 * c2 + 16 * h:64 * c2 + 16 * (h + 1)],
                        start=(h == 0), stop=False)
                    nc.tensor.matmul(SA[:, 64 + h:65 + h], E2,
                                     ones_bf[:, 0:1], start=False,
                                     stop=(h == NH - 1))
                rR = s2.tile([128, 4], F32, tag="rR")
                nc.vector.reciprocal(out=rR, in_=SA[:, 64:68])
                rRb = bass.AP(tensor=rR.tensor, offset=rR.offset,
                              ap=[rR.ap[0], rR.ap[1], [0, 16]])
                nc.vector.tensor_tensor(
                    out=saNp[tp][:, o2:o2 + 64].rearrange(
                        "p (h k) -> p h k", k=16),
                    in0=SA[:, 0:64].rearrange("p (h k) -> p h k", k=16),
                    in1=rRb, op=ALU.mult)
            saNT = s2.tile([128, 128], BF16, tag="saNT", bufs=2)
            xbar(saNT, saNp[tp])
            for c2 in range(2 * tp, min(2 * tp + 2, NCHUNK)):
                o2 = 64 * (c2 % 2)
                x1ps = s2p.tile([128, 64], F32, tag="ps")
                nc.tensor.matmul(x1ps, saNT[o2:o2 + 64, :],
                                 sb["WoR"][o2:o2 + 64, :],
                                 start=True, stop=True)
                x1s = s2.tile([128, 64], F32, tag="x1s")
                nc.vector.tensor_tensor(out=x1s, in0=x1ps,
                                        in1=Xpair[tp][:, o2:o2 + 64],
                                        op=ALU.add)
                if flags["bo"]:
                    nc.vector.tensor_tensor(out=x1s, in0=x1s,
                                            in1=sb["bo_b"], op=ALU.add)
                ln(x1p_t[tp][:, o2:o2 + 64], x1s, flags["ln1"],
                   "ln1g_b", "ln1b_b")
            x1T = s2.tile([128, 128], BF16, tag="x1T", bufs=2)
            xbar(x1T, x1p_t[tp])
            for c2 in range(2 * tp, min(2 * tp + 2, NCHUNK)):
                o2 = 64 * (c2 % 2)
                f1a = s2p.tile([128, 128], F32, tag="ps")
                nc.tensor.matmul(f1a, sb["W1aR"][o2:o2 + 64, :],
                                 x1T[o2:o2 + 64, :], start=True, stop=True)
                fga = s2.tile([128, 128], BF16, tag="fga")
                gelu(fga, f1a, bias=sb["b1a"][:, 0:1])
                f1b = s2p.tile([128, 128], F32, tag="ps")
                nc.tensor.matmul(f1b, sb["W1bR"][o2:o2 + 64, :],
                                 x1T[o2:o2 + 64, :], start=True, stop=True)
                fgb = s2.tile([128, 128], BF16, tag="fgb")
                gelu(fgb, f1b, bias=sb["b1b"][:, 0:1])
                f2 = s2p.tile([128, 64], F32, tag="ps")
                nc.tensor.matmul(f2, fga, sb["W2a"], start=True, stop=False)
                nc.tensor.matmul(f2, fgb, sb["W2b"], start=False, stop=True)
                x2s = s2.tile([128, 64], F32, tag="x2s")
                nc.vector.tensor_tensor(out=x2s, in0=f2,
                                        in1=x1p_t[tp][:, o2:o2 + 64],
                                        op=ALU.add)
                if flags["b2ffn"]:
                    nc.vector.tensor_tensor(out=x2s, in0=x2s,
                                            in1=sb["b2f_b"], op=ALU.add)
                ln(x2p_t[tp][:, o2:o2 + 64], x2s, flags["ln2"],
                   "ln2g_b", "ln2b_b")
            x2T = s2.tile([128, 128], BF16, tag="x2T", bufs=2)
            xbar(x2T, x2p_t[tp])
            for c2 in range(2 * tp, min(2 * tp + 2, NCHUNK)):
                o2 = 64 * (c2 % 2)
                KTp = s2p.tile([64, 128], F32, tag="ps")
                nc.tensor.matmul(KTp, sb["WkclsR"][o2:o2 + 64, :],
                                 x2T[o2:o2 + 64, :], start=True, stop=True)
                nc.scalar.activation(
                    out=KclsT_sb[:, 128 * c2:128 * (c2 + 1)], in_=KTp,
                    func=AF.Identity, bias=sb["bkcls"][0:64, 0:1])
                wvtp = s2p.tile([1, 128], F32, tag="ps")
                nc.tensor.matmul(wvtp, sb["wvecR"][o2:o2 + 64, 0:1],
                                 x2T[o2:o2 + 64, :], start=True, stop=True)
                nc.scalar.activation(out=wvT_sb[:, 128 * c2:128 * (c2 + 1)],
                                     in_=wvtp, func=AF.Copy,
                                     bias=flags["cvh"])
            for c2 in range(2 * tp, min(2 * tp + 2, NCHUNK)):
                SCc = s2p.tile([C, 128], F32, tag="ps")
                nc.tensor.matmul(SCc, sb["CqT"][0:64, :],
                                 KclsT_sb[:, 128 * c2:128 * (c2 + 1)],
                                 start=True, stop=True)
                E2c = s2.tile([C, 128], F32, tag="E2c")
                nc.scalar.activation(out=E2c, in_=SCc, func=AF.Exp)
                wv7 = s2p.tile([C, 128], F32, tag="ps")
                nc.tensor.matmul(wv7, ones7,
                                 wvT_sb[:, 128 * c2:128 * (c2 + 1)],
                                 start=True, stop=True)
                prod = s2.tile([C, 128], F32, tag="prod")
                nc.vector.tensor_tensor(out=prod, in0=E2c, in1=wv7,
                                        op=ALU.mult)
                num = s2.tile([C, 8], F32, tag="num")
                nc.vector.reduce_sum(
                    out=num, in_=prod.rearrange("c (b k) -> c b k", k=16),
                    axis=mybir.AxisListType.X)
                den = s2.tile([C, 8], F32, tag="den")
                nc.vector.reduce_sum(
                    out=den, in_=E2c.rearrange("c (b k) -> c b k", k=16),
                    axis=mybir.AxisListType.X)
                rden = s2.tile([C, 8], F32, tag="rden")
                nc.vector.reciprocal(out=rden, in_=den)
                oc = s2.tile([C, 8], F32, tag="oc")
                nc.vector.tensor_tensor(out=oc, in0=num, in1=rden,
                                        op=ALU.mult)
                nc.vector.tensor_scalar_add(
                    OUT_sb[:, 8 * c2:8 * (c2 + 1)], oc, flags["bh"])

        # ================= stage 1 + interleaved stage 2 =================
        # 16-batch load-groups (2 loads + 2 xbars total): the deadlock guard
        # serializes SWDGE DMAs against DMA-transposes one unit at a time,
        # so fewer/bigger units minimize the serialization hops. Chunk
        # structure (8 batches x 16 slots per 128 partitions) is unchanged.
        GB = 16
        NLG = NG // 2
        hqs = []
        for lg in range(NLG):
            hq = hbp.tile([128, GB, NT * 64], BF16, tag="hq")
            # two 8-batch half-loads per group: the scheduler can slot the
            # first xbar right after group 0's halves, before later loads
            for hf in range(2):
                b0 = GB * lg + 8 * hf
                nc.gpsimd.dma_start(
                    out=hq[:, 8 * hf:8 * (hf + 1), :],
                    in_=hp.ap()[b0:b0 + 8].rearrange(
                        "b (p q) d -> p b (q d)", p=128))
            hqs.append(hq)
        for lg in range(NLG):
            hq = hqs[lg]
            xt = xtp.tile([128, GB * NT // 2, 128], BF16, tag="xt")
            # tiled xbar transpose: chunk 9b+c = [2q*64d, 128 tok]
            nc.sync.dma_start(out=xt, in_=hq, transpose=True)
            if mk_d is not None:
                mk = smp.tile([128, GB, NT], I32, tag="mk")
                nc.sync.dma_start(
                    out=mk,
                    in_=mk_d.ap()[GB * lg:GB * (lg + 1)].rearrange(
                        "b (p q) -> p b q", p=128))
                mkf = smp.tile([128, GB, NT], F32, tag="mkf")
                nc.vector.tensor_copy(out=mkf, in_=mk)
            P8s = [ppp.tile([128, 512], F32, tag="P8", name=f"P8_{lg}_{ch}")
                   for ch in range(2)]
            Pps = [pop.tile([128, 3], F32, tag="Pp", name=f"Pp_{lg}_{ch}")
                   for ch in range(2)]
            for c in range(NCT):
                LG = lgp.tile([128, 32 * GB], F32, tag="LG")
                for b in range(GB):
                    nc.tensor.matmul(LG[:, 32 * b:32 * (b + 1)],
                                     xt[:, NCT * b + c, :], sb["AT2"],
                                     start=True, stop=True)
                E8 = smp.tile([128, 32 * GB], BF16, tag="E8")
                nc.scalar.activation(out=E8, in_=LG, func=AF.Exp)
                if flags["c0"]:
                    ec = sb["expc0_b"]
                    ecb = bass.AP(tensor=ec.tensor, offset=ec.offset,
                                  ap=[ec.ap[0], [0, 2 * GB], ec.ap[1]])
                    nc.vector.tensor_tensor(
                        out=E8.rearrange("p (g k) -> p g k", k=K),
                        in0=E8.rearrange("p (g k) -> p g k", k=K),
                        in1=ecb, op=ALU.mult)
                sig = smp.tile([128, 2 * GB], F32, tag="sig")
                nc.vector.tensor_reduce(
                    out=sig, in_=E8.rearrange("p (g k) -> p g k", k=K),
                    op=ALU.add, axis=mybir.AxisListType.X)
                rsig = smp.tile([128, 2 * GB], F32, tag="rsig")
                nc.vector.reciprocal(out=rsig, in_=sig)
                if mk_d is not None:
                    nc.vector.tensor_tensor(
                        out=rsig.rearrange("p (b q) -> p b q", q=2),
                        in0=rsig.rearrange("p (b q) -> p b q", q=2),
                        in1=mkf[:, :, 2 * c:2 * c + 2], op=ALU.mult)
                # pm stored parity-major [p, qp, b, k] so each parity
                # half-slice is a contiguous [128, 128] stationary operand
                pm8 = smp.tile([128, 2, GB, K], BF16, tag="pm8")
                rsv = rsig.rearrange("p (b q) -> p b q", q=2)
                rb = bass.AP(tensor=rsv.tensor, offset=rsv.offset,
                             ap=[rsv.ap[0], rsv.ap[1], rsv.ap[2], [0, K]])
                nc.vector.tensor_tensor(
                    out=pm8.rearrange("p q b k -> p b q k"),
                    in0=E8.rearrange("p (b q k) -> p b q k", q=2, k=K),
                    in1=rb, op=ALU.mult)
                for ch in range(2):
                    for qp in range(2):
                        st = (c == 0 and qp == 0)
                        sp = (c == NCT - 1 and qp == 1)
                        nc.tensor.matmul(
                            P8s[ch], pm8[:, qp, 8 * ch:8 * (ch + 1), :],
                            hq[:, 8 * ch:8 * (ch + 1),
                               64 * (2 * c + qp):64 * (2 * c + qp + 1)],
                            start=st, stop=sp)
                        nc.tensor.matmul(Pps[ch],
                                         pm8[:, qp, 8 * ch:8 * (ch + 1), :],
                                         sb["posE"][:, 2 * c + qp, :],
                                         start=st, stop=sp)
            # G extraction per chunk: masked accumulate over column blocks
            for ch in range(2):
                Gg = G[2 * lg + ch]
                P8 = P8s[ch]
                nc.vector.tensor_scalar_mul(Gg[:, 0:64], P8[:, 0:64],
                                            sb["mask8"][:, 0:1])
                for b in range(1, 8):
                    nc.vector.scalar_tensor_tensor(
                        out=Gg[:, 0:64], in0=P8[:, 64 * b:64 * (b + 1)],
                        scalar=sb["mask8"][:, b:b + 1], in1=Gg[:, 0:64],
                        op0=ALU.mult, op1=ALU.add)
                nc.vector.tensor_copy(out=Gg[:, 64:67], in_=Pps[ch])
                s2_preamble(2 * lg + ch)
            stage2_pair(lg)
        nc.sync.dma_start(out=out_d.ap().rearrange("b c -> c b"), in_=OUT_sb)


_CACHE = {}
TRACE = False          # test harness can set kernel.TRACE = True
LAST_RESULT = None     # BassKernelResults of the last kernel() call


def _get_program(nbatch, n, use_mask, flags, blob_cols, meta):
    key = (nbatch, n, use_mask, tuple(sorted(
        (k, v) for k, v in flags.items() if isinstance(v, bool))))
    if key not in _CACHE:
        _CACHE[key] = build(nbatch, n, use_mask, flags, blob_cols, meta)
    return _CACHE[key]


def kernel(**inputs):
    from concourse.bass_utils import run_bass_kernel_spmd

    h_pixel = np.ascontiguousarray(np.asarray(inputs["h_pixel"],
                                              dtype=np.float32))
    node_mask = np.ascontiguousarray(np.asarray(inputs["node_mask"],
                                                dtype=np.int32))
    b, n, d = h_pixel.shape
    params = {k: v for k, v in inputs.items()
              if k not in ("h_pixel", "node_mask")}
    blobs, meta, flags = host_prep(params, n=n)
    blob_cols = {BF16: blobs[BF16].shape[1], F32: blobs[F32].shape[1]}
    use_mask = bool(not np.all(node_mask == 1))
    nbatch = b // NCORES
    nc = _get_program(nbatch, n, use_mask, flags, blob_cols, meta)

    in_maps = []
    for core in range(NCORES):
        m = {"hp": h_pixel[core * nbatch:(core + 1) * nbatch],
             "blob_bf": blobs[BF16], "blob_f32": blobs[F32]}
        if use_mask:
            m["mask"] = node_mask[core * nbatch:(core + 1) * nbatch]
        in_maps.append(m)
    kwargs = {}
    if TRACE:
        kwargs["trace"] = True
    res = run_bass_kernel_spmd(nc, in_maps, core_ids=list(range(NCORES)),
                               **kwargs)
    global LAST_RESULT
    LAST_RESULT = res
    out = np.concatenate([r["out"] for r in res.results], axis=0)
    return out.astype(np.float32)


if __name__ == "__main__":
    sys.path.insert(0, "/root/problem")
    import reference
    inputs = {k: np.asarray(v) for k, v in reference.setup_inputs().items()}
    got = kernel(**inputs)
    print("out shape", got.shape)
